# revision 1
# baseline (speedup 1.0000x reference)
"""Trainium2 Bass kernel for nn_DGBasedGaussianKLD.

Math (per reference):
  z[b,s,d] = mean[b,d] + eps[b,s,d]*exp(0.5*logvar[b,d])
  For each chunk c (batch split into nc=4 chunks of agg_size=256) and each
  dim d, with samples j = (b_local, s) (8192 of them) and components
  i = the 256 chunk rows:
    log_q_ij = -0.5*((z_j - mu_i)^2 * e^{-lv_i} + lv_i + LOG2PI)
             = a_i*z_j^2 + b_i*z_j + c_i          (quadratic in z_j)
    q_j  = mean_i exp(log_q_ij)
    logq[c,d] = mean_j log q_j
    logp[c,d] = mean_j -0.5*(z_j^2 + LOG2PI)
  out = sum_d mean_c (logq - logp)

Sharding: 8 cores = 4 chunks x 2 sample-halves. Each core handles all 32
dims ("pairs") for its (chunk, half): 32 pairs x (256 comps x 4096 samples)
= 33.5M pairwise exps per core.

Device kernel per core (all heavy work):
  - PE: per j-tile of 128 samples, K=3 matmul  E = [z^2; z; 1]^T @ [a; b; c]
    -> PSUM [128 j, 256 i]  (float32r: full-rate fp32-ish matmul)
  - ACT: exp over [128, 2048] PSUM -> SBUF bf16
  - DVE: tensor_tensor_reduce per component-group: q_j = sum_i exp(E)
    (fp32 accumulate)
  - ACT: Ln(q) + accumulate -> per-partition partial sums [128,1]
Host: builds X/W operands (cheap: ~1M FLOPs vs 268M exps on device),
computes logp and final reduction.
"""

import numpy as np

LOG2PI = float(np.log(2.0 * np.pi))
N_CORES = 8

# Hardcoded problem geometry (see spec): batch=1024, dim_z=32, n_samples=32,
# agg_size=256 -> nchunks=4; per core: 32 pairs x 32 j-tiles(128) x 256 comps.
BATCH, DIM_Z, N_SAMPLES, AGG = 1024, 32, 32, 256
NCHUNK = BATCH // AGG           # 4
JH = AGG // 2 * N_SAMPLES       # 4096 samples per core (half chunk)
NPAIR = DIM_Z                   # 32 pairs (dims) per core
NJT = JH // 128                 # 32 j-tiles per pair
ROUNDS = NPAIR * NJT // 8       # 128 rounds of 8 j-tiles

_PROG = None


def _build_program():
    import concourse.bacc as bacc
    import concourse.tile as tile
    from concourse import mybir

    AF = mybir.ActivationFunctionType
    ALU = mybir.AluOpType
    f32 = mybir.dt.float32
    bf16 = mybir.dt.bfloat16

    nc = bacc.Bacc(
        "TRN2", target_bir_lowering=False, debug=False, num_devices=N_CORES
    )
    # Split-bf16 operands: E = z2h*ah + z2l*ah + z2h*al + zh*bh + zl*bh
    #                        + zh*bl + 1*ch + 1*cl  (fp32-grade accuracy,
    # bf16 matmul speed: 1 cy/col + fast weight load).
    # K is padded to 128 (w rows 8-127 are zero) so the weight load is a
    # standard full 128x128 bf16 load (fast-weight-load eligible).
    # x8[p, r, jt*128+jj] = row r in [z2h, z2l, z2h, zh, zl, zh, 1, 1].
    x8_d = nc.dram_tensor("x8", [NPAIR, 8, JH], bf16, kind="ExternalInput").ap()
    # w8 rows 0-7: [ah, ah, al, bh, bh, bl, ch, cl]; rows 8-127 zero.
    w8_d = nc.dram_tensor("w8", [128, NPAIR * AGG], bf16, kind="ExternalInput").ap()
    out_d = nc.dram_tensor("out", [128, 2], f32, kind="ExternalOutput").ap()

    with tile.TileContext(nc) as tc:
        with (
            tc.tile_pool(name="w", bufs=1) as wp,
            tc.tile_pool(name="x", bufs=1) as xp,
            tc.tile_pool(name="ps", bufs=2, space="PSUM") as pp,
            tc.tile_pool(name="ex", bufs=4) as ep,
            tc.tile_pool(name="tree", bufs=6) as tp,
            tc.tile_pool(name="misc", bufs=1) as mp,
        ):
            q_buf = mp.tile([128, ROUNDS * 8], f32)
            logs = mp.tile([128, ROUNDS * 8], f32)
            res = mp.tile([128, 2], f32)
            # warm the exp/ln activation table before any data arrives
            warm = mp.tile([128, 1], f32)
            nc.vector.memset(warm[:], 1.0)
            nc.scalar.activation(logs[:, 2:3], warm[:], AF.Exp)
            # static ping-pong X buffers; rows 8-127 zeroed once (W rows are
            # zero there too, but garbage NaN/Inf x 0 would poison PSUM).
            xbufs = [
                xp.tile([128, JH], bf16, tag=f"xs{i}", name=f"xs{i}")
                for i in range(2)
            ]
            nc.vector.memset(xbufs[0][:, :], 0.0)
            nc.gpsimd.memset(xbufs[1][:, :], 0.0)
            # prefetch the first two pairs' X before the bulk W transfer so
            # the first matmuls aren't queued behind 2MB of weights
            nc.sync.dma_start(xbufs[0][0:8, :], x8_d[0])
            nc.sync.dma_start(xbufs[1][0:8, :], x8_d[1])
            w_all = wp.tile([128, NPAIR * AGG], bf16)
            # chunked so the first pairs' weights arrive quickly
            for qd in range(8):
                cw = NPAIR * AGG // 8
                nc.sync.dma_start(
                    w_all[:, qd * cw : (qd + 1) * cw],
                    w8_d[:, qd * cw : (qd + 1) * cw],
                )

            x_t = None
            for t in range(ROUNDS):
                p, quarter = divmod(t, NJT // 8)
                if quarter == 0:
                    x_t = xbufs[p % 2]
                    if p >= 2:
                        nc.sync.dma_start(x_t[0:8, :], x8_d[p])
                ps = pp.tile([128, 8 * AGG], f32)
                for k in range(8):
                    jt = quarter * 8 + k
                    nc.tensor.matmul(
                        ps[:, k * AGG : (k + 1) * AGG],
                        lhsT=x_t[:, jt * 128 : (jt + 1) * 128],
                        rhs=w_all[:, p * AGG : (p + 1) * AGG],
                        start=True,
                        stop=True,
                    )
                ex = ep.tile([128, 8 * AGG], bf16)
                nc.scalar.activation(ex[:], ps[:], AF.Exp)
                # per-group sum over 256 comps: bf16 pairwise-add tree
                # (2x DVE mode) down to 32/group, then one 1x reduce.
                h = ex
                w = AGG
                for _ in range(3):  # 256 -> 32 per group
                    hn = tp.tile([128, 8 * (w // 2)], bf16, tag="tree")
                    hg = h[:].rearrange("p (g k) -> p g k", g=8)
                    hng = hn[:].rearrange("p (g k) -> p g k", g=8)
                    nc.vector.tensor_tensor(
                        hng[:, :, :],
                        hg[:, :, 0 : w // 2],
                        hg[:, :, w // 2 : w],
                        ALU.add,
                    )
                    h, w = hn, w // 2
                nc.vector.tensor_reduce(
                    q_buf[:, t * 8 : (t + 1) * 8],
                    h[:].rearrange("p (g k) -> p g k", g=8),
                    axis=mybir.AxisListType.X,
                    op=ALU.add,
                )
            half = ROUNDS * 8 // 2
            nc.scalar.activation(
                logs[:, 0:half], q_buf[:, 0:half], AF.Ln, accum_out=res[:, 0:1]
            )
            nc.scalar.activation(
                logs[:, half:], q_buf[:, half:], AF.Ln, accum_out=res[:, 1:2]
            )
            nc.sync.dma_start(out_d[:], res[:])

    nc.compile()
    return nc


def _get_program():
    global _PROG
    if _PROG is None:
        _PROG = _build_program()
    return _PROG


def _reference_numpy(mean, logvar, eps, n_samples, agg_size):
    """Exact fallback for unexpected geometry (never hit for the spec case)."""
    batch, dim_z = mean.shape
    if batch % agg_size != 0:
        agg_size = batch
    nchunks = batch // agg_size
    std = np.exp(0.5 * logvar)
    z = mean[:, None, :] + eps * std[:, None, :]
    z2 = z.reshape(nchunks, agg_size * n_samples, dim_z)
    mu = mean.reshape(nchunks, agg_size, 1, dim_z)
    lv = logvar.reshape(nchunks, agg_size, 1, dim_z)
    log_q = -0.5 * (
        (z2[:, None, :, :] - mu) ** 2 * np.exp(-lv) + lv + LOG2PI
    )
    logq = np.log(np.exp(log_q).mean(axis=1)).mean(axis=1)
    logp = (-0.5 * (z2**2 + LOG2PI)).mean(axis=1)
    return np.float32((logq - logp).mean(axis=0).sum(axis=-1))


def _prep_in_maps(mean, logvar, eps):
    import ml_dtypes

    bf = ml_dtypes.bfloat16

    def split_bf16(v):
        hi = v.astype(bf)
        lo = (v - hi.astype(np.float32)).astype(bf)
        return hi, lo

    # Host prep (f32, same op order as reference for z).
    std = np.exp(np.float32(0.5) * logvar)
    z = mean[:, None, :] + eps * std[:, None, :]  # [1024, 32, 32] f32

    e_nlv = np.exp(-logvar)  # [1024, 32]
    a = np.float32(-0.5) * e_nlv
    b = mean * e_nlv
    c = np.float32(-0.5) * (mean * mean * e_nlv + logvar + np.float32(LOG2PI))

    in_maps = []
    for core in range(N_CORES):
        ch, half = divmod(core, 2)
        b0 = ch * AGG + half * (AGG // 2)
        # z for this core: [128 b, 32 s, 32 d] -> per pair d: flat j = b*32+s
        zc = z[b0 : b0 + AGG // 2]  # [128, 32, 32]
        zp = np.ascontiguousarray(zc.transpose(2, 0, 1).reshape(NPAIR, JH))
        z2 = zp * zp
        z2h, z2l = split_bf16(z2)
        zh, zl = split_bf16(zp)
        ones = np.ones_like(zp, dtype=bf)
        rows = [z2h, z2l, z2h, zh, zl, zh, ones, ones]  # [NPAIR, JH] each
        x8 = np.stack(rows, axis=1)  # [NPAIR, 8, JH]
        r0, r1 = ch * AGG, (ch + 1) * AGG

        def pairs_flat(v):
            return v[r0:r1].T.reshape(-1)  # [32 pairs x 256 comps]

        ah, al = split_bf16(pairs_flat(a))
        bh, bl = split_bf16(pairs_flat(b))
        chd, cl = split_bf16(pairs_flat(c))
        w8rows = np.stack([ah, ah, al, bh, bh, bl, chd, cl])
        w8 = np.zeros((128, NPAIR * AGG), dtype=bf)
        w8[0:8] = w8rows
        in_maps.append({"x8": x8, "w8": w8})
    return in_maps, z


def kernel(mean, logvar, eps, n_samples, agg_size):
    from concourse.bass_utils import run_bass_kernel_spmd

    mean = np.asarray(mean, dtype=np.float32)
    logvar = np.asarray(logvar, dtype=np.float32)
    eps = np.asarray(eps, dtype=np.float32)
    n_samples = int(n_samples)
    agg_size = int(agg_size)

    if (mean.shape, eps.shape, n_samples, agg_size) != (
        (BATCH, DIM_Z),
        (BATCH, N_SAMPLES, DIM_Z),
        N_SAMPLES,
        AGG,
    ):
        return _reference_numpy(mean, logvar, eps, n_samples, agg_size)

    in_maps, z = _prep_in_maps(mean, logvar, eps)

    nc = _get_program()
    res = run_bass_kernel_spmd(nc, in_maps, list(range(N_CORES)))
    global _LAST_RESULTS
    _LAST_RESULTS = res

    # T1 = sum over all (c,d,j) of ln(sum_i exp(E_ij))
    t1 = np.float64(0.0)
    for core in range(N_CORES):
        t1 += res.results[core]["out"].astype(np.float64).sum()

    nsamp = AGG * N_SAMPLES  # 8192
    logq_sum = t1 / nsamp - NCHUNK * DIM_Z * np.log(np.float64(AGG))
    # logp: host (cheap, from the same z the device used)
    z64 = z.astype(np.float64).reshape(NCHUNK, nsamp, DIM_Z)
    z2mean = (z64**2).mean(axis=1)  # [nc, dim_z]
    logp_sum = (-0.5 * (z2mean + LOG2PI)).sum()
    return np.float32((logq_sum - logp_sum) / NCHUNK)



# revision 2
# speedup vs baseline: 11.5937x; 11.5937x over previous
"""Trainium2 Bass kernel for nn_DGBasedGaussianKLD.

Math (per reference):
  z[b,s,d] = mean[b,d] + eps[b,s,d]*exp(0.5*logvar[b,d])
  For each chunk c (batch split into nc=4 chunks of agg_size=256) and each
  dim d, with samples j = (b_local, s) (8192 of them) and components
  i = the 256 chunk rows:
    log_q_ij = -0.5*((z_j - mu_i)^2 * e^{-lv_i} + lv_i + LOG2PI)
    q_j  = mean_i exp(log_q_ij)
    logq[c,d] = mean_j log q_j
    logp[c,d] = mean_j -0.5*(z_j^2 + LOG2PI)
  out = sum_d mean_c (logq - logp)

Quadrature reformulation (device work 64x smaller than direct eval):
  For fixed (c,d), f(x) = ln sum_i exp(a_i x^2 + b_i x + c_i) is a smooth
  1-D function.  mean_j f(z_j) is computed by evaluating f on a uniform
  B=128-point grid spanning [min z, max z] and combining with Catmull-Rom
  cubic-interpolation weights accumulated from the samples (host-side
  bincounts).  Measured quadrature rel-err on the final scalar: ~3e-6
  (device bf16 numerics dominate at ~1e-4).

The per-(c,d) affine map x = xmid + s*u (u in [-1,1] shared grid) is folded
into the coefficients so the grid operand X is shared by all pairs/cores:
    a' = a s^2,  b' = (2 a xmid + b) s,  c' = a xmid^2 + b xmid + c

Sharding: 128 (c,d) pairs over 8 cores = 16 pairs/core
(core k -> chunk k//2, dims (k%2)*16 .. +16).

Device kernel per core:
  - PE: 8 matmuls, K=8 (split-bf16 rows), N=512: E = X^T W -> PSUM
    [128 grid pts, 16 pairs x 256 comps]
  - ACT: exp over [128, 2048] PSUM -> SBUF bf16 (x2 halves)
  - DVE: bf16 pairwise-add tree 256->32 per pair + 1x tensor_reduce
    -> q[128 pts, 16 pairs] f32
Host: builds X/W operands + quadrature weights (~1M flops), computes
ln q, weighted sums, logp, and the final scalar in float64.
"""

import numpy as np

LOG2PI = float(np.log(2.0 * np.pi))
N_CORES = 8

# Hardcoded problem geometry (see spec): batch=1024, dim_z=32, n_samples=32,
# agg_size=256 -> nchunks=4.
BATCH, DIM_Z, N_SAMPLES, AGG = 1024, 32, 32, 256
NCHUNK = BATCH // AGG           # 4
B = 128                         # grid points per (chunk, dim) pair
NPC = 16                        # pairs per core (4*32 / 8)
NSAMP = AGG * N_SAMPLES         # 8192 samples per chunk

_PROG = None


def _build_program():
    import concourse.bacc as bacc
    import concourse.tile as tile
    from concourse import mybir

    AF = mybir.ActivationFunctionType
    ALU = mybir.AluOpType
    f32 = mybir.dt.float32
    bf16 = mybir.dt.bfloat16

    nc = bacc.Bacc(
        "TRN2", target_bir_lowering=False, debug=False, num_devices=N_CORES
    )
    # Split-bf16 operands (fp32-grade accuracy, bf16 matmul speed):
    # E = u2h*ah + u2l*ah + u2h*al + uh*bh + ul*bh + uh*bl + ch + cl
    # K=8 contraction: no padding, no SBUF zeroing needed.
    # x8 rows r: [u2h, u2l, u2h, uh, ul, uh, 1, 1] at the 128 grid points.
    x8_d = nc.dram_tensor("x8", [8, B], bf16, kind="ExternalInput").ap()
    # w8 rows: [ah, ah, al, bh, bh, bl, ch, cl]; col = pair*256 + comp.
    w8_d = nc.dram_tensor("w8", [8, NPC * AGG], bf16, kind="ExternalInput").ap()
    out_d = nc.dram_tensor("out", [B, NPC], f32, kind="ExternalOutput").ap()

    with tile.TileContext(nc) as tc:
        with (
            tc.tile_pool(name="io", bufs=1) as iop,
            tc.tile_pool(name="ps", bufs=2, space="PSUM") as pp,
            tc.tile_pool(name="ex", bufs=2) as ep,
            tc.tile_pool(name="tree", bufs=6) as tp,
            tc.tile_pool(name="misc", bufs=1) as mp,
        ):
            xs = iop.tile([8, B], bf16)
            ws = iop.tile([8, NPC * AGG], bf16)
            q = mp.tile([B, NPC], f32)
            nc.sync.dma_start(xs[:], x8_d[:])
            nc.sync.dma_start(ws[:], w8_d[:])
            # warm the exp activation table while the DMAs are in flight
            warm = mp.tile([128, 1], f32)
            nc.vector.memset(warm[:], 1.0)
            nc.scalar.activation(warm[:], warm[:], AF.Exp)

            for half in range(2):  # 8 pairs per half
                ps = pp.tile([128, 8 * AGG], f32)  # 4 PSUM banks
                for k in range(4):  # 2 pairs per matmul (N=512 = 1 bank)
                    col = half * 8 * AGG + k * 512
                    nc.tensor.matmul(
                        ps[:, k * 512 : (k + 1) * 512],
                        lhsT=xs[:, :],
                        rhs=ws[:, col : col + 512],
                        start=True,
                        stop=True,
                    )
                ex = ep.tile([128, 8 * AGG], bf16)
                nc.scalar.activation(ex[:], ps[:], AF.Exp)
                # per-pair sum over 256 comps: bf16 pairwise-add tree
                # (2x DVE mode) down to 32/pair, then one 1x reduce.
                h = ex
                w = AGG
                for _ in range(3):  # 256 -> 32 per pair
                    hn = tp.tile([128, 8 * (w // 2)], bf16, tag="tree")
                    hg = h[:].rearrange("p (g k) -> p g k", g=8)
                    hng = hn[:].rearrange("p (g k) -> p g k", g=8)
                    nc.vector.tensor_tensor(
                        hng[:, :, :],
                        hg[:, :, 0 : w // 2],
                        hg[:, :, w // 2 : w],
                        ALU.add,
                    )
                    h, w = hn, w // 2
                nc.vector.tensor_reduce(
                    q[:, half * 8 : (half + 1) * 8],
                    h[:].rearrange("p (g k) -> p g k", g=8),
                    axis=mybir.AxisListType.X,
                    op=ALU.add,
                )
            nc.sync.dma_start(out_d[:], q[:])

    nc.compile()
    return nc


def _get_program():
    global _PROG
    if _PROG is None:
        _PROG = _build_program()
    return _PROG


def _reference_numpy(mean, logvar, eps, n_samples, agg_size):
    """Exact fallback for unexpected geometry (never hit for the spec case)."""
    batch, dim_z = mean.shape
    if batch % agg_size != 0:
        agg_size = batch
    nchunks = batch // agg_size
    std = np.exp(0.5 * logvar)
    z = mean[:, None, :] + eps * std[:, None, :]
    z2 = z.reshape(nchunks, agg_size * n_samples, dim_z)
    mu = mean.reshape(nchunks, agg_size, 1, dim_z)
    lv = logvar.reshape(nchunks, agg_size, 1, dim_z)
    log_q = -0.5 * (
        (z2[:, None, :, :] - mu) ** 2 * np.exp(-lv) + lv + LOG2PI
    )
    logq = np.log(np.exp(log_q).mean(axis=1)).mean(axis=1)
    logp = (-0.5 * (z2**2 + LOG2PI)).mean(axis=1)
    return np.float32((logq - logp).mean(axis=0).sum(axis=-1))


def _split_bf16(v):
    import ml_dtypes

    bf = ml_dtypes.bfloat16
    hi = v.astype(np.float32).astype(bf)
    lo = (v.astype(np.float32) - hi.astype(np.float32)).astype(bf)
    return hi, lo


def _prep(mean, logvar, eps):
    """Host prep: z, grid ranges, folded split-bf16 coefficients, weights."""
    import ml_dtypes

    bf = ml_dtypes.bfloat16

    # z with the same f32 op order as the reference
    std = np.exp(np.float32(0.5) * logvar)
    z = mean[:, None, :] + eps * std[:, None, :]  # [1024, 32, 32] f32
    z2 = z.reshape(NCHUNK, NSAMP, DIM_Z)

    x0 = z2.min(axis=1).astype(np.float64)  # [nc, dim_z]
    x1 = z2.max(axis=1).astype(np.float64)
    xmid = 0.5 * (x0 + x1)
    s = 0.5 * (x1 - x0)

    mu = mean.astype(np.float64).reshape(NCHUNK, AGG, DIM_Z)
    lv = logvar.astype(np.float64).reshape(NCHUNK, AGG, DIM_Z)
    e = np.exp(-lv)
    a = -0.5 * e                                    # [nc, agg, dim_z]
    b = mu * e
    c = -0.5 * (mu * mu * e + lv + LOG2PI)
    # fold x = xmid + s*u into the quadratic (u in [-1,1])
    a2 = a * (s * s)[:, None, :]
    b2 = (2.0 * a * xmid[:, None, :] + b) * s[:, None, :]
    c2 = (a * xmid[:, None, :] + b) * xmid[:, None, :] + c

    # shared grid operand
    u = -1.0 + 2.0 * np.arange(B) / (B - 1)         # f64 [128]
    u2h, u2l = _split_bf16(u * u)
    uh, ul = _split_bf16(u)
    ones = np.ones(B, dtype=bf)
    x8 = np.stack([u2h, u2l, u2h, uh, ul, uh, ones, ones])  # [8, 128]

    ah, al = _split_bf16(a2)  # [nc, agg, dim_z] each
    bh, bl = _split_bf16(b2)
    ch, cl = _split_bf16(c2)

    in_maps = []
    for core in range(N_CORES):
        cidx, hd = divmod(core, 2)
        d0 = hd * NPC
        # rows [8], cols [pair, comp] flattened: pair-major
        def pf(v):
            return np.ascontiguousarray(
                v[cidx, :, d0 : d0 + NPC].T
            ).reshape(-1).astype(bf)

        w8 = np.stack([pf(ah), pf(ah), pf(al), pf(bh), pf(bh), pf(bl),
                       pf(ch), pf(cl)])  # [8, NPC*AGG]
        in_maps.append({"x8": x8, "w8": w8})

    # Catmull-Rom quadrature weights per (c,d): [nc, dim_z, B]
    wq = np.zeros((NCHUNK, DIM_Z, B))
    h = (x1 - x0) / (B - 1)                          # [nc, dim_z]
    for ci in range(NCHUNK):
        for d in range(DIM_Z):
            zd = z2[ci, :, d].astype(np.float64)
            t = (zd - x0[ci, d]) / h[ci, d]
            i = np.clip(np.floor(t).astype(np.int64), 0, B - 2)
            fr = t - i
            im1 = np.clip(i - 1, 0, B - 1)
            ip2 = np.clip(i + 2, 0, B - 1)
            f2 = fr * fr
            f3 = f2 * fr
            wq[ci, d] += np.bincount(im1, -0.5 * fr + f2 - 0.5 * f3, minlength=B)
            wq[ci, d] += np.bincount(i, 1.0 - 2.5 * f2 + 1.5 * f3, minlength=B)
            wq[ci, d] += np.bincount(i + 1, 0.5 * fr + 2.0 * f2 - 1.5 * f3,
                                     minlength=B)
            wq[ci, d] += np.bincount(ip2, -0.5 * f2 + 0.5 * f3, minlength=B)

    return in_maps, z2, wq


def kernel(mean, logvar, eps, n_samples, agg_size):
    from concourse.bass_utils import run_bass_kernel_spmd

    mean = np.asarray(mean, dtype=np.float32)
    logvar = np.asarray(logvar, dtype=np.float32)
    eps = np.asarray(eps, dtype=np.float32)
    n_samples = int(n_samples)
    agg_size = int(agg_size)

    if (mean.shape, eps.shape, n_samples, agg_size) != (
        (BATCH, DIM_Z),
        (BATCH, N_SAMPLES, DIM_Z),
        N_SAMPLES,
        AGG,
    ):
        return _reference_numpy(mean, logvar, eps, n_samples, agg_size)

    in_maps, z2, wq = _prep(mean, logvar, eps)

    nc = _get_program()
    res = run_bass_kernel_spmd(nc, in_maps, list(range(N_CORES)))
    global _LAST_RESULTS
    _LAST_RESULTS = res

    # logq[c,d] = sum_b wq[c,d,b] * (ln q[c,d,b] - ln 256) / 8192
    logq = np.zeros((NCHUNK, DIM_Z))
    for core in range(N_CORES):
        cidx, hd = divmod(core, 2)
        qv = res.results[core]["out"].astype(np.float64)  # [B, NPC]
        f = np.log(np.maximum(qv, 1e-300)) - np.log(256.0)
        w = wq[cidx, hd * NPC : (hd + 1) * NPC]           # [NPC, B]
        logq[cidx, hd * NPC : (hd + 1) * NPC] = (
            np.where(w != 0.0, w * f.T, 0.0).sum(axis=1) / NSAMP
        )

    z64 = z2.astype(np.float64)
    logp = (-0.5 * (z64**2 + LOG2PI)).mean(axis=1)        # [nc, dim_z]
    return np.float32(((logq - logp).mean(axis=0)).sum())


# revision 4
# speedup vs baseline: 12.2940x; 1.0604x over previous
"""Trainium2 Bass kernel for nn_DGBasedGaussianKLD.

Math (per reference):
  z[b,s,d] = mean[b,d] + eps[b,s,d]*exp(0.5*logvar[b,d])
  For each chunk c (batch split into nc=4 chunks of agg_size=256) and each
  dim d, with samples j = (b_local, s) (8192 of them) and components
  i = the 256 chunk rows:
    log_q_ij = -0.5*((z_j - mu_i)^2 * e^{-lv_i} + lv_i + LOG2PI)
    q_j  = mean_i exp(log_q_ij)
    logq[c,d] = mean_j log q_j
    logp[c,d] = mean_j -0.5*(z_j^2 + LOG2PI)
  out = sum_d mean_c (logq - logp)

Quadrature reformulation (device work 64x smaller than direct eval):
  For fixed (c,d), f(x) = ln sum_i exp(a_i x^2 + b_i x + c_i) is a smooth
  1-D function.  mean_j f(z_j) is computed by evaluating f on a uniform
  B=128-point grid spanning [min z, max z] and combining with Catmull-Rom
  cubic-interpolation weights accumulated from the samples (host-side
  bincounts).  Measured quadrature rel-err on the final scalar: ~3e-6
  (device bf16 numerics dominate at ~1e-4).

The per-(c,d) affine map x = xmid + s*u (u in [-1,1] shared grid) is folded
into the coefficients so the grid operand X is shared by all pairs/cores:
    a' = a s^2,  b' = (2 a xmid + b) s,  c' = a xmid^2 + b xmid + c

Sharding: 128 (c,d) pairs over 8 cores = 16 pairs/core
(core k -> chunk k//2, dims (k%2)*16 .. +16).

Device kernel per core:
  - PE: 8 matmuls, K=8 (split-bf16 rows), N=512: E = X^T W -> PSUM
    [128 grid pts, 16 pairs x 256 comps]
  - ACT: exp over [128, 2048] PSUM -> SBUF bf16 (x2 halves)
  - DVE: bf16 pairwise-add tree 256->32 per pair + 1x tensor_reduce
    -> q[128 pts, 16 pairs] f32
Host: builds X/W operands + quadrature weights (~1M flops), computes
ln q, weighted sums, logp, and the final scalar in float64.
"""

import numpy as np

LOG2PI = float(np.log(2.0 * np.pi))
N_CORES = 8

# Hardcoded problem geometry (see spec): batch=1024, dim_z=32, n_samples=32,
# agg_size=256 -> nchunks=4.
BATCH, DIM_Z, N_SAMPLES, AGG = 1024, 32, 32, 256
NCHUNK = BATCH // AGG           # 4
B = 128                         # grid points per (chunk, dim) pair
NPC = 16                        # pairs per core (4*32 / 8)
NSAMP = AGG * N_SAMPLES         # 8192 samples per chunk

_PROG = None


def _build_program():
    import concourse.bacc as bacc
    import concourse.tile as tile
    from concourse import mybir

    AF = mybir.ActivationFunctionType
    ALU = mybir.AluOpType
    f32 = mybir.dt.float32
    bf16 = mybir.dt.bfloat16

    nc = bacc.Bacc(
        "TRN2", target_bir_lowering=False, debug=False, num_devices=N_CORES
    )
    # Split-bf16 operands (fp32-grade accuracy, bf16 matmul speed):
    # E = u2h*ah + u2l*ah + u2h*al + uh*bh + ul*bh + uh*bl + ch + cl
    # K=8 contraction: no padding, no SBUF zeroing needed.
    # Matmuls alternate between PE row groups 0 and 64 so weight loads
    # overlap in-flight matmuls (concurrent 32-row subarrays).  The W
    # columns are partitioned between the groups and the grid operand X
    # (rows [u2h, u2l, u2h, uh, ul, uh, 1, 1]) is appended to each
    # group's W so one DMA per group delivers everything.
    # w8x rows 8g:8g+8, cols: [4 blocks x 512 (2 pairs each) | 128 X].
    w8x_d = nc.dram_tensor(
        "w8x", [16, NPC * AGG // 2 + B], bf16, kind="ExternalInput"
    ).ap()
    out_d = nc.dram_tensor("out", [B, NPC], f32, kind="ExternalOutput").ap()
    WCOL = NPC * AGG // 2  # 2048 W columns per group

    with tile.TileContext(nc) as tc:
        with (
            tc.tile_pool(name="io", bufs=1) as iop,
            tc.tile_pool(name="ps", bufs=2, space="PSUM") as pp,
            tc.tile_pool(name="ex", bufs=2) as ep,
            tc.tile_pool(name="tree", bufs=6) as tp,
            tc.tile_pool(name="misc", bufs=1) as mp,
        ):
            ws = iop.tile([72, WCOL + B], bf16)
            q = mp.tile([B, NPC], f32)
            nc.sync.dma_start(ws[0:8, :], w8x_d[0:8, :])
            nc.sync.dma_start(ws[64:72, :], w8x_d[8:16, :])
            # warm the exp activation table while the DMAs are in flight
            warm = mp.tile([128, 1], f32)
            warm2 = mp.tile([128, 1], f32)
            nc.vector.memset(warm[:], 1.0)
            nc.scalar.activation(warm2[:], warm[:], AF.Exp)

            for half in range(2):  # 8 pairs per half
                ps = pp.tile([128, 8 * AGG], f32)  # 4 PSUM banks
                for s in range(4):  # 2 pairs per matmul (N=512 = 1 bank)
                    g = s % 2          # PE row group (partition 64*g)
                    blk = 2 * half + s // 2
                    nc.tensor.matmul(
                        ps[:, s * 512 : (s + 1) * 512],
                        lhsT=ws[64 * g : 64 * g + 8, WCOL : WCOL + B],
                        rhs=ws[64 * g : 64 * g + 8, blk * 512 : (blk + 1) * 512],
                        start=True,
                        stop=True,
                        tile_position=(64 * g, 0),
                    )
                ex = ep.tile([128, 8 * AGG], bf16)
                nc.scalar.activation(ex[:], ps[:], AF.Exp)
                # per-pair sum over 256 comps: bf16 pairwise-add tree
                # (2x DVE mode) down to 32/pair, then one 1x reduce.
                h = ex
                w = AGG
                for _ in range(3):  # 256 -> 32 per pair
                    hn = tp.tile([128, 8 * (w // 2)], bf16, tag="tree")
                    hg = h[:].rearrange("p (g k) -> p g k", g=8)
                    hng = hn[:].rearrange("p (g k) -> p g k", g=8)
                    nc.vector.tensor_tensor(
                        hng[:, :, :],
                        hg[:, :, 0 : w // 2],
                        hg[:, :, w // 2 : w],
                        ALU.add,
                    )
                    h, w = hn, w // 2
                nc.vector.tensor_reduce(
                    q[:, half * 8 : (half + 1) * 8],
                    h[:].rearrange("p (g k) -> p g k", g=8),
                    axis=mybir.AxisListType.X,
                    op=ALU.add,
                )
            nc.sync.dma_start(out_d[:], q[:])

    nc.compile()
    return nc


def _get_program():
    global _PROG
    if _PROG is None:
        _PROG = _build_program()
    return _PROG


def _reference_numpy(mean, logvar, eps, n_samples, agg_size):
    """Exact fallback for unexpected geometry (never hit for the spec case)."""
    batch, dim_z = mean.shape
    if batch % agg_size != 0:
        agg_size = batch
    nchunks = batch // agg_size
    std = np.exp(0.5 * logvar)
    z = mean[:, None, :] + eps * std[:, None, :]
    z2 = z.reshape(nchunks, agg_size * n_samples, dim_z)
    mu = mean.reshape(nchunks, agg_size, 1, dim_z)
    lv = logvar.reshape(nchunks, agg_size, 1, dim_z)
    log_q = -0.5 * (
        (z2[:, None, :, :] - mu) ** 2 * np.exp(-lv) + lv + LOG2PI
    )
    logq = np.log(np.exp(log_q).mean(axis=1)).mean(axis=1)
    logp = (-0.5 * (z2**2 + LOG2PI)).mean(axis=1)
    return np.float32((logq - logp).mean(axis=0).sum(axis=-1))


def _split_bf16(v):
    import ml_dtypes

    bf = ml_dtypes.bfloat16
    hi = v.astype(np.float32).astype(bf)
    lo = (v.astype(np.float32) - hi.astype(np.float32)).astype(bf)
    return hi, lo


def _prep(mean, logvar, eps):
    """Host prep: z, grid ranges, folded split-bf16 coefficients, weights."""
    import ml_dtypes

    bf = ml_dtypes.bfloat16

    # z with the same f32 op order as the reference
    std = np.exp(np.float32(0.5) * logvar)
    z = mean[:, None, :] + eps * std[:, None, :]  # [1024, 32, 32] f32
    z2 = z.reshape(NCHUNK, NSAMP, DIM_Z)

    x0 = z2.min(axis=1).astype(np.float64)  # [nc, dim_z]
    x1 = z2.max(axis=1).astype(np.float64)
    xmid = 0.5 * (x0 + x1)
    s = 0.5 * (x1 - x0)

    mu = mean.astype(np.float64).reshape(NCHUNK, AGG, DIM_Z)
    lv = logvar.astype(np.float64).reshape(NCHUNK, AGG, DIM_Z)
    e = np.exp(-lv)
    a = -0.5 * e                                    # [nc, agg, dim_z]
    b = mu * e
    c = -0.5 * (mu * mu * e + lv + LOG2PI)
    # fold x = xmid + s*u into the quadratic (u in [-1,1])
    a2 = a * (s * s)[:, None, :]
    b2 = (2.0 * a * xmid[:, None, :] + b) * s[:, None, :]
    c2 = (a * xmid[:, None, :] + b) * xmid[:, None, :] + c

    # shared grid operand
    u = -1.0 + 2.0 * np.arange(B) / (B - 1)         # f64 [128]
    u2h, u2l = _split_bf16(u * u)
    uh, ul = _split_bf16(u)
    ones = np.ones(B, dtype=bf)
    x8 = np.stack([u2h, u2l, u2h, uh, ul, uh, ones, ones])  # [8, 128]

    ah, al = _split_bf16(a2)  # [nc, agg, dim_z] each
    bh, bl = _split_bf16(b2)
    ch, cl = _split_bf16(c2)

    in_maps = []
    for core in range(N_CORES):
        cidx, hd = divmod(core, 2)
        d0 = hd * NPC
        # rows [8], dims [pair, comp]
        def pf(v):
            return np.ascontiguousarray(v[cidx, :, d0 : d0 + NPC].T).astype(bf)

        w8 = np.stack([pf(ah), pf(ah), pf(al), pf(bh), pf(bh), pf(bl),
                       pf(ch), pf(cl)])  # [8, NPC, AGG]
        # group/block layout: group g, block b holds pairs
        # p0 = 8*(b//2) + 4*(b%2) + 2g and p0+1; X appended per group.
        w8x = np.zeros((16, NPC * AGG // 2 + B), dtype=bf)
        for g in range(2):
            for b_ in range(4):
                p0 = 8 * (b_ // 2) + 4 * (b_ % 2) + 2 * g
                w8x[8 * g : 8 * g + 8, b_ * 512 : b_ * 512 + 256] = w8[:, p0]
                w8x[8 * g : 8 * g + 8, b_ * 512 + 256 : (b_ + 1) * 512] = (
                    w8[:, p0 + 1]
                )
            w8x[8 * g : 8 * g + 8, NPC * AGG // 2 :] = x8
        in_maps.append({"w8x": w8x})

    # Catmull-Rom quadrature weights per (c,d): [nc, dim_z, B]
    wq = np.zeros((NCHUNK, DIM_Z, B))
    h = (x1 - x0) / (B - 1)                          # [nc, dim_z]
    for ci in range(NCHUNK):
        for d in range(DIM_Z):
            zd = z2[ci, :, d].astype(np.float64)
            t = (zd - x0[ci, d]) / h[ci, d]
            i = np.clip(np.floor(t).astype(np.int64), 0, B - 2)
            fr = t - i
            im1 = np.clip(i - 1, 0, B - 1)
            ip2 = np.clip(i + 2, 0, B - 1)
            f2 = fr * fr
            f3 = f2 * fr
            wq[ci, d] += np.bincount(im1, -0.5 * fr + f2 - 0.5 * f3, minlength=B)
            wq[ci, d] += np.bincount(i, 1.0 - 2.5 * f2 + 1.5 * f3, minlength=B)
            wq[ci, d] += np.bincount(i + 1, 0.5 * fr + 2.0 * f2 - 1.5 * f3,
                                     minlength=B)
            wq[ci, d] += np.bincount(ip2, -0.5 * f2 + 0.5 * f3, minlength=B)

    return in_maps, z2, wq


def kernel(mean, logvar, eps, n_samples, agg_size):
    from concourse.bass_utils import run_bass_kernel_spmd

    mean = np.asarray(mean, dtype=np.float32)
    logvar = np.asarray(logvar, dtype=np.float32)
    eps = np.asarray(eps, dtype=np.float32)
    n_samples = int(n_samples)
    agg_size = int(agg_size)

    if (mean.shape, eps.shape, n_samples, agg_size) != (
        (BATCH, DIM_Z),
        (BATCH, N_SAMPLES, DIM_Z),
        N_SAMPLES,
        AGG,
    ):
        return _reference_numpy(mean, logvar, eps, n_samples, agg_size)

    in_maps, z2, wq = _prep(mean, logvar, eps)

    nc = _get_program()
    res = run_bass_kernel_spmd(nc, in_maps, list(range(N_CORES)))
    global _LAST_RESULTS
    _LAST_RESULTS = res

    # logq[c,d] = sum_b wq[c,d,b] * (ln q[c,d,b] - ln 256) / 8192
    logq = np.zeros((NCHUNK, DIM_Z))
    for core in range(N_CORES):
        cidx, hd = divmod(core, 2)
        qv = res.results[core]["out"].astype(np.float64)  # [B, NPC]
        f = np.log(np.maximum(qv, 1e-300)) - np.log(256.0)
        w = wq[cidx, hd * NPC : (hd + 1) * NPC]           # [NPC, B]
        logq[cidx, hd * NPC : (hd + 1) * NPC] = (
            np.where(w != 0.0, w * f.T, 0.0).sum(axis=1) / NSAMP
        )

    z64 = z2.astype(np.float64)
    logp = (-0.5 * (z64**2 + LOG2PI)).mean(axis=1)        # [nc, dim_z]
    return np.float32(((logq - logp).mean(axis=0)).sum())
